# revision 13
# baseline (speedup 1.0000x reference)
"""nn_GATLayer Trainium2 kernel: 8-core SPMD Bass/Tile implementation.

kernel(**inputs) takes the FULL inputs (feat [100000,128] f32, W, attn_l,
attn_r, bias, src/dst [1600000] i32) and returns the FULL output
[100000, 4, 16] f32.

Strategy (dst-sharded, collective-free, node-partitioned edge slots):
  Host: sort nodes by in-degree, deal them round-robin into 98 "levels" x
  8 cores x 128 partition slots so every core sees the same per-level max
  in-degree D_l. Per core, build a slot table [128, sum(D_l)] of h_aug row
  indices: column off_l+s, partition p holds the source row of the s-th
  incoming edge of the node at (level l, partition p); empty slots point
  at a pad row whose el is -30 (exp ~ 0). All float math runs on device.
  Device (per core, same SPMD program, fully unrolled for overlap):
    Phase A: h_aug = feat @ [W | wl | wr] for ALL nodes (replicated),
      where el = feat@wl, er = feat@wr fold the attention dot-products
      into the projection matmul. Rows [h(64) | el(4) | er(4)] bf16,
      stored swizzled (node at featT col c -> row (c%128)*792 + c//128)
      so phase A writes are 2304B-contiguous per partition.
    Phase B (per level, python-unrolled):
      per-slot indirect DMAs gather the block's 128*D source rows
      [128, D, 72] (one offset per partition per instruction); er of the
      block's own nodes comes from one batched upfront DMA; logits =
      leaky_relu(el_src + er_dst) in f32; ex = exp -> bf16 alongside
      msg = h_src * ex; a halving add-tree over the slot axis (f32
      accumulation) produces numerator [128,64] and denominator [128,4]
      in one pass; out = num/max(den,eps)+bias into an SBUF staging
      tile, written to DRAM once at the end.
  Softmax max-subtraction is skipped: alpha = ex/sum(ex) is shift-
  invariant and logits are bounded (~|10|) for this distribution.
"""

import numpy as np
from contextlib import ExitStack

import ml_dtypes

import concourse.bass as bass
import concourse.tile as tile
from concourse import bacc, mybir
from concourse.bass import ds
from concourse.bass_utils import run_bass_kernel_spmd

F32 = mybir.dt.float32
BF16 = mybir.dt.bfloat16
I32 = mybir.dt.int32

P = 128
N_CORES = 8
NB = 98                    # destination levels (blocks per core)
D_OUT = 16
HEADS = 4
HD = HEADS * D_OUT         # 64
HAUG = HD + 2 * HEADS      # 72

NPC = NB * P               # 12544 nodes per core
NPOS = N_CORES * NPC       # 100352 padded node positions
JW = 792                   # h_aug j-columns per partition (784 used + pad)
PAD_ROW = 784              # h_aug row of the pad row (p=0, j=784)
NROWS = P * JW             # 101376 h_aug rows

N_NODES = 100000
IN_DIM = 128

LAST_RESULTS = None        # BassKernelResults of the most recent run


def build_host_data(feat, src, dst):
    """Degree-sorted node layout + per-core slot tables (int math only)."""
    deg = np.zeros(NPOS, dtype=np.int64)
    np.add.at(deg, dst, 1)
    order = np.argsort(-deg, kind="stable")      # rank -> node position
    rank = np.empty(NPOS, dtype=np.int64)
    rank[order] = np.arange(NPOS)                # node -> rank (= featT col)

    # per-level slot width: max degree in level = degree of first rank
    D = deg[order[::P * N_CORES]].astype(np.int64)          # [NB]
    assert D.shape == (NB,)
    off = np.concatenate([[0], np.cumsum(D)])               # [NB+1]
    S = int(off[-1])

    # featT [128, NPOS] bf16, col = rank of node
    featT = np.zeros((IN_DIM, NPOS), dtype=ml_dtypes.bfloat16)
    featT[:, rank[:feat.shape[0]]] = feat.T.astype(ml_dtypes.bfloat16)

    r_src = rank[src]
    r_dst = rank[dst]
    # h_aug row of a node at featT col c: (c % 128)*JW + c//128
    idx_src = ((r_src % P) * JW + r_src // P).astype(np.int32)

    lvl = r_dst // (P * N_CORES)
    core = (r_dst % (P * N_CORES)) // P
    part = r_dst % P

    # slot index within the destination node: stable order among its edges
    e_order = np.argsort(r_dst, kind="stable")
    cnt = np.bincount(r_dst, minlength=NPOS)
    starts = np.concatenate([[0], np.cumsum(cnt)[:-1]])
    slot = np.empty(len(src), dtype=np.int64)
    slot[e_order] = np.arange(len(src)) - starts[r_dst[e_order]]

    col = off[lvl] + slot
    tabs = []
    for c in range(N_CORES):
        m = core == c
        tab = np.full((P, S), PAD_ROW, dtype=np.int32)
        tab[part[m], col[m]] = idx_src[m]
        tabs.append(tab)

    # output mapping: node n -> (core, row l*128+p)
    out_core = ((rank[:N_NODES] % (P * N_CORES)) // P).astype(np.int64)
    out_row = ((rank[:N_NODES] // (P * N_CORES)) * P
               + rank[:N_NODES] % P).astype(np.int64)

    return dict(featT=featT, tabs=tabs, D=tuple(int(x) for x in D),
                out_core=out_core, out_row=out_row, S=S)


def build_program(D, S, den_eps=1e-6):
    """D: per-level slot counts (same for all cores); S = sum(D)."""
    DCAP = max(max(D), 1)
    nc = bacc.Bacc("TRN2", target_bir_lowering=False, debug=False,
                   num_devices=N_CORES)

    featT_d = nc.dram_tensor("featT", [IN_DIM, NPOS], BF16, kind="ExternalInput")
    w_d = nc.dram_tensor("W", [IN_DIM, HD], F32, kind="ExternalInput")
    al_d = nc.dram_tensor("attn_l", [HEADS, D_OUT], F32, kind="ExternalInput")
    ar_d = nc.dram_tensor("attn_r", [HEADS, D_OUT], F32, kind="ExternalInput")
    bias_d = nc.dram_tensor("bias", [HD], F32, kind="ExternalInput")
    tab_d = nc.dram_tensor("tab", [P, S], I32, kind="ExternalInput")

    hA_d = nc.dram_tensor("h_aug", [NROWS, HAUG], BF16, kind="Internal")
    out_d = nc.dram_tensor("out", [NPC, HD], F32, kind="ExternalOutput")

    hA_w = hA_d[:].rearrange("(p j) c -> p (j c)", j=JW)     # [128, JW*72]
    # [p, b, (k c)] view: row p*792 + 8b + k  (b < 99)
    hA_r = hA_d[:].rearrange("(p b k) c -> p b (k c)", b=JW // 8, k=8)

    with tile.TileContext(nc) as tc, ExitStack() as ctx:
        cpool = ctx.enter_context(tc.tile_pool(name="const", bufs=1))
        fpool = ctx.enter_context(tc.tile_pool(name="ft", bufs=5))
        hpool = ctx.enter_context(tc.tile_pool(name="hb", bufs=5))
        pspool = ctx.enter_context(tc.tile_pool(name="psA", bufs=6, space="PSUM"))
        epool = ctx.enter_context(tc.tile_pool(name="edge", bufs=3))
        mpool = ctx.enter_context(tc.tile_pool(name="msg", bufs=2))
        apool = ctx.enter_context(tc.tile_pool(name="acc", bufs=2))
        spool = ctx.enter_context(tc.tile_pool(name="small", bufs=4))

        pid = nc.sync.partition_id()

        # ---- constants / parameter prep (one-time) ----
        ones_row = cpool.tile([1, P], F32)
        nc.vector.memset(ones_row[:], 1.0)

        def pe_broadcast(row_ap, width):
            ps = pspool.tile([P, width], F32)
            nc.tensor.matmul(out=ps[:], lhsT=ones_row[:], rhs=row_ap,
                             start=True, stop=True)
            t = cpool.tile([P, width], F32)
            nc.vector.tensor_copy(t[:], ps[:])
            return t

        al_row = cpool.tile([1, HD], F32)
        nc.sync.dma_start(al_row[:], al_d[:].rearrange("h d -> (h d)").unsqueeze(0))
        ar_row = cpool.tile([1, HD], F32)
        nc.sync.dma_start(ar_row[:], ar_d[:].rearrange("h d -> (h d)").unsqueeze(0))
        b_row = cpool.tile([1, HD], F32)
        nc.sync.dma_start(b_row[:], bias_d[:].unsqueeze(0))

        al_b = pe_broadcast(al_row[:], HD)
        ar_b = pe_broadcast(ar_row[:], HD)
        bias_b = pe_broadcast(b_row[:], HD)

        w_aug = cpool.tile([P, HAUG], F32)
        nc.sync.dma_start(w_aug[:, 0:HD], w_d[:, :])
        tmp = cpool.tile([P, HD], F32)
        nc.vector.tensor_tensor(out=tmp[:], in0=w_aug[:, 0:HD], in1=al_b[:],
                                op=mybir.AluOpType.mult)
        nc.vector.tensor_reduce(
            out=w_aug[:, HD:HD + HEADS],
            in_=tmp[:].rearrange("p (h d) -> p h d", d=D_OUT),
            axis=mybir.AxisListType.X, op=mybir.AluOpType.add)
        nc.vector.tensor_tensor(out=tmp[:], in0=w_aug[:, 0:HD], in1=ar_b[:],
                                op=mybir.AluOpType.mult)
        nc.vector.tensor_reduce(
            out=w_aug[:, HD + HEADS:HAUG],
            in_=tmp[:].rearrange("p (h d) -> p h d", d=D_OUT),
            axis=mybir.AxisListType.X, op=mybir.AluOpType.add)
        w_augb = cpool.tile([P, HAUG], BF16)
        nc.vector.tensor_copy(w_augb[:], w_aug[:])

        # pad row: h = 0, el = -30 (exp -> ~0), er = 0
        pad_t = cpool.tile([1, HAUG], BF16)
        nc.vector.memset(pad_t[:], 0.0)
        nc.vector.memset(pad_t[:, HD:HD + HEADS], -30.0)
        nc.sync.dma_start(hA_d[ds(PAD_ROW, 1), :], pad_t[:])

        # whole slot table staged once (offsets into it are compile-time)
        stab = cpool.tile([P, S], I32)
        nc.sync.dma_start(stab[:], tab_d[:, :])

        # ---- Phase A: projection (replicated over all nodes) ----
        PROJ_G = 16
        n_groups = NPOS // (PROJ_G * P)          # 49
        for gi in range(n_groups):
            ft = fpool.tile([P, PROJ_G * P], BF16, tag="ft")
            nc.sync.dma_start(ft[:], featT_d[:, ds(gi * (PROJ_G * P), PROJ_G * P)])
            hb = hpool.tile([P, PROJ_G * HAUG], BF16, tag="hb")
            for j in range(PROJ_G):
                ps = pspool.tile([P, HAUG], F32)
                nc.tensor.matmul(out=ps[:], lhsT=ft[:, j * P:(j + 1) * P],
                                 rhs=w_augb[:], start=True, stop=True)
                # split psum->sbuf cast copies across Act and DVE so the
                # copy chain doesn't serialize phase A on one engine
                if j % 2 == 0:
                    nc.scalar.copy(hb[:, j * HAUG:(j + 1) * HAUG], ps[:])
                else:
                    nc.vector.tensor_copy(hb[:, j * HAUG:(j + 1) * HAUG], ps[:])
            nc.sync.dma_start(hA_w[:, ds(gi * (PROJ_G * HAUG), PROJ_G * HAUG)],
                              hb[:])

        tc.strict_bb_all_engine_barrier()

        # er of this core's nodes for ALL levels in one DMA: [128, 98*4]
        er_all = cpool.tile([P, NB * HEADS], BF16)
        nc.sync.dma_start(
            er_all[:].rearrange("p (b c) -> p b c", c=HEADS),
            hA_r[:, 0:NB, ds(pid * HAUG + HD + HEADS, HEADS)])

        # output staging: all levels, one DRAM write at the end
        out_all = cpool.tile([P, NB * HD], F32)

        # ---- Phase B: per-level edge aggregation (fully unrolled) ----
        off = 0
        for b in range(NB):
            D_b = D[b]
            ob = out_all[:, b * HD:(b + 1) * HD]
            if D_b == 0:
                nc.vector.tensor_copy(ob, bias_b[:])
                continue

            hsrc = epool.tile([P, DCAP * HAUG], BF16, tag="hsrc")
            for dd in range(D_b):
                nc.gpsimd.indirect_dma_start(
                    out=hsrc[:, dd * HAUG:(dd + 1) * HAUG],
                    out_offset=None,
                    in_=hA_d[:],
                    in_offset=bass.IndirectOffsetOnAxis(
                        ap=stab[:, off + dd:off + dd + 1], axis=0),
                )
            h3 = hsrc[:].rearrange("p (s c) -> p s c", c=HAUG)

            lg = spool.tile([P, DCAP * HEADS], F32, tag="lg")
            lg3 = lg[:].rearrange("p (s h) -> p s h", h=HEADS)
            nc.vector.tensor_tensor(
                out=lg3[:, 0:D_b, :],
                in0=h3[:, 0:D_b, HD:HD + HEADS],
                in1=er_all[:, b * HEADS:(b + 1) * HEADS]
                    .unsqueeze(1).broadcast_to([P, D_b, HEADS]),
                op=mybir.AluOpType.add)
            lk = spool.tile([P, DCAP * HEADS], F32, tag="lk")
            nc.vector.tensor_scalar_mul(lk[:, 0:D_b * HEADS],
                                        lg[:, 0:D_b * HEADS], 0.2)
            nc.vector.tensor_tensor(out=lk[:, 0:D_b * HEADS],
                                    in0=lk[:, 0:D_b * HEADS],
                                    in1=lg[:, 0:D_b * HEADS],
                                    op=mybir.AluOpType.max)

            MW = HD + HEADS   # 68
            msg = mpool.tile([P, DCAP * MW], BF16, tag="msg")
            m3 = msg[:].rearrange("p (s c) -> p s c", c=MW)
            nc.scalar.activation(
                out=m3[:, 0:D_b, HD:MW],
                in_=lk[:, 0:D_b * HEADS].rearrange("p (s h) -> p s h", h=HEADS),
                func=mybir.ActivationFunctionType.Exp)
            nc.vector.tensor_tensor(
                out=m3[:, 0:D_b, 0:HD].rearrange("p s (h d) -> p s h d", d=D_OUT),
                in0=h3[:, 0:D_b, 0:HD].rearrange("p s (h d) -> p s h d", d=D_OUT),
                in1=m3[:, 0:D_b, HD:MW].unsqueeze(3).broadcast_to(
                    [P, D_b, HEADS, D_OUT]),
                op=mybir.AluOpType.mult)

            # halving add-tree over slots: bf16 msg -> f32 acc
            accf = apool.tile([P, ((DCAP + 1) // 2) * MW], F32, tag="accf")
            a3 = accf[:].rearrange("p (s c) -> p s c", c=MW)
            n = D_b
            if n == 1:
                nc.vector.tensor_copy(a3[:, 0:1, :], m3[:, 0:1, :])
            else:
                h = n // 2
                nc.vector.tensor_tensor(out=a3[:, 0:h, :],
                                        in0=m3[:, 0:h, :],
                                        in1=m3[:, h:2 * h, :],
                                        op=mybir.AluOpType.add)
                if n % 2:
                    nc.vector.tensor_copy(a3[:, h:h + 1, :],
                                          m3[:, 2 * h:2 * h + 1, :])
                n = h + (n % 2)
            while n > 1:
                if n % 2:
                    nc.vector.tensor_tensor(out=a3[:, 0:1, :],
                                            in0=a3[:, 0:1, :],
                                            in1=a3[:, n - 1:n, :],
                                            op=mybir.AluOpType.add)
                    n -= 1
                h = n // 2
                nc.vector.tensor_tensor(out=a3[:, 0:h, :],
                                        in0=a3[:, 0:h, :],
                                        in1=a3[:, h:2 * h, :],
                                        op=mybir.AluOpType.add)
                n = h

            den = spool.tile([P, HEADS], F32, tag="den")
            nc.vector.tensor_scalar_max(
                den[:], a3[:, 0:1, HD:MW].rearrange("p s c -> p (s c)"), den_eps)
            rec = spool.tile([P, HEADS], F32, tag="rec")
            nc.vector.reciprocal(rec[:], den[:])

            nc.vector.tensor_tensor(
                out=ob.rearrange("p (h d) -> p h d", d=D_OUT),
                in0=a3[:, 0:1, 0:HD].rearrange("p s (h d) -> p (s h) d", d=D_OUT),
                in1=rec[:].unsqueeze(2).broadcast_to([P, HEADS, D_OUT]),
                op=mybir.AluOpType.mult)
            nc.vector.tensor_tensor(out=ob, in0=ob, in1=bias_b[:],
                                    op=mybir.AluOpType.add)
            off += D_b

        # one output write for everything: row b*128+p <- out_all[p, b, :]
        nc.sync.dma_start(
            out_d[:].rearrange("(b p) c -> p b c", p=P),
            out_all[:].rearrange("p (b c) -> p b c", c=HD))

    nc.compile()
    return nc


_PROGRAM_CACHE = {}


def run(feat, W, attn_l, attn_r, bias, src, dst):
    global LAST_RESULTS
    feat = np.asarray(feat, dtype=np.float32)
    src = np.asarray(src, dtype=np.int32)
    dst = np.asarray(dst, dtype=np.int32)

    host = build_host_data(feat, src, dst)

    key = (host["D"], host["S"])
    if key not in _PROGRAM_CACHE:
        _PROGRAM_CACHE[key] = build_program(list(host["D"]), host["S"])
    nc = _PROGRAM_CACHE[key]

    in_maps = []
    for c in range(N_CORES):
        in_maps.append({
            "featT": host["featT"],
            "W": np.asarray(W, dtype=np.float32),
            "attn_l": np.asarray(attn_l, dtype=np.float32),
            "attn_r": np.asarray(attn_r, dtype=np.float32),
            "bias": np.asarray(bias, dtype=np.float32),
            "tab": host["tabs"][c],
        })

    res = run_bass_kernel_spmd(nc, in_maps, core_ids=list(range(N_CORES)))
    LAST_RESULTS = res
    outs = np.stack([res.results[c]["out"] for c in range(N_CORES)], axis=0)
    return outs[host["out_core"], host["out_row"]]


def kernel(feat, W, attn_l, attn_r, bias, src, dst):
    out = run(feat, W, attn_l, attn_r, bias, src, dst)
    return out.reshape(N_NODES, HEADS, D_OUT).astype(np.float32)


# revision 15
# speedup vs baseline: 1.1521x; 1.1521x over previous
"""nn_GATLayer Trainium2 kernel: 8-core SPMD Bass/Tile implementation.

kernel(**inputs) takes the FULL inputs (feat [100000,128] f32, W, attn_l,
attn_r, bias, src/dst [1600000] i32) and returns the FULL output
[100000, 4, 16] f32.

Strategy (dst-sharded, collective-free, node-partitioned edge slots):
  Host: sort nodes by in-degree, deal them round-robin into 98 "levels" x
  8 cores x 128 partition slots so every core sees the same per-level max
  in-degree D_l. Per core, build a slot table [128, sum(D_l)] of h_aug row
  indices: column off_l+s, partition p holds the source row of the s-th
  incoming edge of the node at (level l, partition p); empty slots point
  at a pad row whose el is -30 (exp ~ 0). All float math runs on device.
  Device (per core, same SPMD program, fully unrolled for overlap):
    Phase A: h_aug = feat @ [W | wl | wr] for ALL nodes (replicated),
      where el = feat@wl, er = feat@wr fold the attention dot-products
      into the projection matmul. Rows [h(64) | el(4) | er(4)] bf16,
      stored swizzled (node at featT col c -> row (c%128)*792 + c//128)
      so phase A writes are 2304B-contiguous per partition.
    Phase B (per level, python-unrolled):
      per-slot indirect DMAs gather the block's 128*D source rows
      [128, D, 72] (one offset per partition per instruction); er of the
      block's own nodes comes from one batched upfront DMA; logits =
      leaky_relu(el_src + er_dst) in f32; ex = exp -> bf16 alongside
      msg = h_src * ex; a halving add-tree over the slot axis (f32
      accumulation) produces numerator [128,64] and denominator [128,4]
      in one pass; out = num/max(den,eps)+bias into an SBUF staging
      tile, written to DRAM once at the end.
  Softmax max-subtraction is skipped: alpha = ex/sum(ex) is shift-
  invariant and logits are bounded (~|10|) for this distribution.
"""

import numpy as np
from contextlib import ExitStack

import ml_dtypes

import concourse.bass as bass
import concourse.tile as tile
from concourse import bacc, mybir
from concourse.bass import ds
from concourse.bass_utils import run_bass_kernel_spmd

F32 = mybir.dt.float32
BF16 = mybir.dt.bfloat16
I32 = mybir.dt.int32

P = 128
N_CORES = 8
NB = 98                    # destination levels (blocks per core)
D_OUT = 16
HEADS = 4
HD = HEADS * D_OUT         # 64
HAUG = HD + 2 * HEADS      # 72

NPC = NB * P               # 12544 nodes per core
NPOS = N_CORES * NPC       # 100352 padded node positions
JW = 792                   # h_aug j-columns per partition (784 used + pad)
PAD_ROW = 784              # h_aug row of the pad row (p=0, j=784)
NROWS = P * JW             # 101376 h_aug rows

N_NODES = 100000
IN_DIM = 128

LAST_RESULTS = None        # BassKernelResults of the most recent run


def build_host_data(feat, src, dst):
    """Degree-sorted node layout + per-core slot tables (int math only)."""
    deg = np.zeros(NPOS, dtype=np.int64)
    np.add.at(deg, dst, 1)
    order = np.argsort(-deg, kind="stable")      # rank -> node position
    rank = np.empty(NPOS, dtype=np.int64)
    rank[order] = np.arange(NPOS)                # node -> rank (= featT col)

    # per-level slot width: max degree in level = degree of first rank
    D = deg[order[::P * N_CORES]].astype(np.int64)          # [NB]
    assert D.shape == (NB,)
    off = np.concatenate([[0], np.cumsum(D)])               # [NB+1]
    S = int(off[-1])

    # featT [128, NPOS] bf16, col = rank of node
    featT = np.zeros((IN_DIM, NPOS), dtype=ml_dtypes.bfloat16)
    featT[:, rank[:feat.shape[0]]] = feat.T.astype(ml_dtypes.bfloat16)

    r_src = rank[src]
    r_dst = rank[dst]
    # h_aug row of a node at featT col c: (c % 128)*JW + c//128
    idx_src = ((r_src % P) * JW + r_src // P).astype(np.int32)

    lvl = r_dst // (P * N_CORES)
    core = (r_dst % (P * N_CORES)) // P
    part = r_dst % P

    # slot index within the destination node: stable order among its edges
    e_order = np.argsort(r_dst, kind="stable")
    cnt = np.bincount(r_dst, minlength=NPOS)
    starts = np.concatenate([[0], np.cumsum(cnt)[:-1]])
    slot = np.empty(len(src), dtype=np.int64)
    slot[e_order] = np.arange(len(src)) - starts[r_dst[e_order]]

    col = off[lvl] + slot
    tabs = []
    for c in range(N_CORES):
        m = core == c
        tab = np.full((P, S), PAD_ROW, dtype=np.int32)
        tab[part[m], col[m]] = idx_src[m]
        tabs.append(tab)

    # output mapping: node n -> (core, row l*128+p)
    out_core = ((rank[:N_NODES] % (P * N_CORES)) // P).astype(np.int64)
    out_row = ((rank[:N_NODES] // (P * N_CORES)) * P
               + rank[:N_NODES] % P).astype(np.int64)

    return dict(featT=featT, tabs=tabs, D=tuple(int(x) for x in D),
                out_core=out_core, out_row=out_row, S=S)


def build_program(D, S, den_eps=1e-6):
    """D: per-level slot counts (same for all cores); S = sum(D)."""
    DCAP = max(max(D), 1)
    nc = bacc.Bacc("TRN2", target_bir_lowering=False, debug=False,
                   num_devices=N_CORES)

    featT_d = nc.dram_tensor("featT", [IN_DIM, NPOS], BF16, kind="ExternalInput")
    w_d = nc.dram_tensor("W", [IN_DIM, HD], F32, kind="ExternalInput")
    al_d = nc.dram_tensor("attn_l", [HEADS, D_OUT], F32, kind="ExternalInput")
    ar_d = nc.dram_tensor("attn_r", [HEADS, D_OUT], F32, kind="ExternalInput")
    bias_d = nc.dram_tensor("bias", [HD], F32, kind="ExternalInput")
    tab_d = nc.dram_tensor("tab", [P, S], I32, kind="ExternalInput")

    hA_d = nc.dram_tensor("h_aug", [NROWS, HAUG], BF16, kind="Internal")
    out_d = nc.dram_tensor("out", [NPC, HD], F32, kind="ExternalOutput")

    hA_w = hA_d[:].rearrange("(p j) c -> p (j c)", j=JW)     # [128, JW*72]
    # [p, b, (k c)] view: row p*792 + 8b + k  (b < 99)
    hA_r = hA_d[:].rearrange("(p b k) c -> p b (k c)", b=JW // 8, k=8)

    with tile.TileContext(nc) as tc, ExitStack() as ctx:
        cpool = ctx.enter_context(tc.tile_pool(name="const", bufs=1))
        fpool = ctx.enter_context(tc.tile_pool(name="ft", bufs=2))
        hpool = ctx.enter_context(tc.tile_pool(name="hb", bufs=3))
        pspool = ctx.enter_context(tc.tile_pool(name="psA", bufs=4, space="PSUM"))
        epool = ctx.enter_context(tc.tile_pool(name="edge", bufs=3))
        mpool = ctx.enter_context(tc.tile_pool(name="msg", bufs=2))
        apool = ctx.enter_context(tc.tile_pool(name="acc", bufs=2))
        spool = ctx.enter_context(tc.tile_pool(name="small", bufs=4))

        pid = nc.sync.partition_id()

        # ---- constants / parameter prep (one-time) ----
        ones_row = cpool.tile([1, P], F32)
        nc.vector.memset(ones_row[:], 1.0)

        def pe_broadcast(row_ap, width):
            ps = pspool.tile([P, width], F32)
            nc.tensor.matmul(out=ps[:], lhsT=ones_row[:], rhs=row_ap,
                             start=True, stop=True)
            t = cpool.tile([P, width], F32)
            nc.vector.tensor_copy(t[:], ps[:])
            return t

        al_row = cpool.tile([1, HD], F32)
        nc.sync.dma_start(al_row[:], al_d[:].rearrange("h d -> (h d)").unsqueeze(0))
        ar_row = cpool.tile([1, HD], F32)
        nc.sync.dma_start(ar_row[:], ar_d[:].rearrange("h d -> (h d)").unsqueeze(0))
        b_row = cpool.tile([1, HD], F32)
        nc.sync.dma_start(b_row[:], bias_d[:].unsqueeze(0))

        al_b = pe_broadcast(al_row[:], HD)
        ar_b = pe_broadcast(ar_row[:], HD)
        bias_b = pe_broadcast(b_row[:], HD)

        w_aug = cpool.tile([P, HAUG], F32)
        nc.sync.dma_start(w_aug[:, 0:HD], w_d[:, :])
        tmp = cpool.tile([P, HD], F32)
        nc.vector.tensor_tensor(out=tmp[:], in0=w_aug[:, 0:HD], in1=al_b[:],
                                op=mybir.AluOpType.mult)
        nc.vector.tensor_reduce(
            out=w_aug[:, HD:HD + HEADS],
            in_=tmp[:].rearrange("p (h d) -> p h d", d=D_OUT),
            axis=mybir.AxisListType.X, op=mybir.AluOpType.add)
        nc.vector.tensor_tensor(out=tmp[:], in0=w_aug[:, 0:HD], in1=ar_b[:],
                                op=mybir.AluOpType.mult)
        nc.vector.tensor_reduce(
            out=w_aug[:, HD + HEADS:HAUG],
            in_=tmp[:].rearrange("p (h d) -> p h d", d=D_OUT),
            axis=mybir.AxisListType.X, op=mybir.AluOpType.add)
        w_augb = cpool.tile([P, HAUG], BF16)
        nc.vector.tensor_copy(w_augb[:], w_aug[:])

        # pad row: h = 0, el = -30 (exp -> ~0), er = 0
        pad_t = cpool.tile([1, HAUG], BF16)
        nc.vector.memset(pad_t[:], 0.0)
        nc.vector.memset(pad_t[:, HD:HD + HEADS], -30.0)
        nc.sync.dma_start(hA_d[ds(PAD_ROW, 1), :], pad_t[:])

        # whole slot table staged once (offsets into it are compile-time)
        stab = cpool.tile([P, S], I32)
        nc.sync.dma_start(stab[:], tab_d[:, :])

        # ---- Phase A: projection (replicated over all nodes) ----
        PROJ_G = 16
        FT_B = 7                                 # groups per featT read
        n_groups = NPOS // (PROJ_G * P)          # 49
        for bi in range(n_groups // FT_B):
            ftb = fpool.tile([P, FT_B * PROJ_G * P], BF16, tag="ft")
            nc.sync.dma_start(
                ftb[:], featT_d[:, ds(bi * (FT_B * PROJ_G * P), FT_B * PROJ_G * P)])
            for gj in range(FT_B):
                gi = bi * FT_B + gj
                hb = hpool.tile([P, PROJ_G * HAUG], BF16, tag="hb")
                for j in range(PROJ_G):
                    ps = pspool.tile([P, HAUG], F32)
                    nc.tensor.matmul(
                        out=ps[:],
                        lhsT=ftb[:, (gj * PROJ_G + j) * P:(gj * PROJ_G + j + 1) * P],
                        rhs=w_augb[:], start=True, stop=True)
                    # split psum->sbuf cast copies across Act and DVE so the
                    # copy chain doesn't serialize phase A on one engine
                    if j % 2 == 0:
                        nc.scalar.copy(hb[:, j * HAUG:(j + 1) * HAUG], ps[:])
                    else:
                        nc.vector.tensor_copy(hb[:, j * HAUG:(j + 1) * HAUG], ps[:])
                nc.sync.dma_start(hA_w[:, ds(gi * (PROJ_G * HAUG), PROJ_G * HAUG)],
                                  hb[:])

        tc.strict_bb_all_engine_barrier()

        # er of this core's nodes for ALL levels in one DMA: [128, 98*4]
        er_all = cpool.tile([P, NB * HEADS], BF16)
        nc.sync.dma_start(
            er_all[:].rearrange("p (b c) -> p b c", c=HEADS),
            hA_r[:, 0:NB, ds(pid * HAUG + HD + HEADS, HEADS)])

        # output staging: all levels, one DRAM write at the end
        out_all = cpool.tile([P, NB * HD], F32)

        # ---- Phase B: per-level edge aggregation (fully unrolled) ----
        off = 0
        for b in range(NB):
            D_b = D[b]
            ob = out_all[:, b * HD:(b + 1) * HD]
            if D_b == 0:
                nc.vector.tensor_copy(ob, bias_b[:])
                continue

            hsrc = epool.tile([P, DCAP * HAUG], BF16, tag="hsrc")
            for dd in range(D_b):
                nc.gpsimd.indirect_dma_start(
                    out=hsrc[:, dd * HAUG:(dd + 1) * HAUG],
                    out_offset=None,
                    in_=hA_d[:],
                    in_offset=bass.IndirectOffsetOnAxis(
                        ap=stab[:, off + dd:off + dd + 1], axis=0),
                )
            h3 = hsrc[:].rearrange("p (s c) -> p s c", c=HAUG)

            lg = spool.tile([P, DCAP * HEADS], F32, tag="lg")
            lg3 = lg[:].rearrange("p (s h) -> p s h", h=HEADS)
            nc.vector.tensor_tensor(
                out=lg3[:, 0:D_b, :],
                in0=h3[:, 0:D_b, HD:HD + HEADS],
                in1=er_all[:, b * HEADS:(b + 1) * HEADS]
                    .unsqueeze(1).broadcast_to([P, D_b, HEADS]),
                op=mybir.AluOpType.add)
            lk = spool.tile([P, DCAP * HEADS], F32, tag="lk")
            nc.vector.tensor_scalar_mul(lk[:, 0:D_b * HEADS],
                                        lg[:, 0:D_b * HEADS], 0.2)
            nc.vector.tensor_tensor(out=lk[:, 0:D_b * HEADS],
                                    in0=lk[:, 0:D_b * HEADS],
                                    in1=lg[:, 0:D_b * HEADS],
                                    op=mybir.AluOpType.max)

            MW = HD + HEADS   # 68
            msg = mpool.tile([P, DCAP * MW], BF16, tag="msg")
            m3 = msg[:].rearrange("p (s c) -> p s c", c=MW)
            nc.scalar.activation(
                out=m3[:, 0:D_b, HD:MW],
                in_=lk[:, 0:D_b * HEADS].rearrange("p (s h) -> p s h", h=HEADS),
                func=mybir.ActivationFunctionType.Exp)
            nc.vector.tensor_tensor(
                out=m3[:, 0:D_b, 0:HD].rearrange("p s (h d) -> p s h d", d=D_OUT),
                in0=h3[:, 0:D_b, 0:HD].rearrange("p s (h d) -> p s h d", d=D_OUT),
                in1=m3[:, 0:D_b, HD:MW].unsqueeze(3).broadcast_to(
                    [P, D_b, HEADS, D_OUT]),
                op=mybir.AluOpType.mult)

            # halving add-tree over slots: bf16 msg -> f32 acc
            accf = apool.tile([P, ((DCAP + 1) // 2) * MW], F32, tag="accf")
            a3 = accf[:].rearrange("p (s c) -> p s c", c=MW)
            n = D_b
            if n == 1:
                nc.vector.tensor_copy(a3[:, 0:1, :], m3[:, 0:1, :])
            else:
                h = n // 2
                nc.vector.tensor_tensor(out=a3[:, 0:h, :],
                                        in0=m3[:, 0:h, :],
                                        in1=m3[:, h:2 * h, :],
                                        op=mybir.AluOpType.add)
                if n % 2:
                    nc.vector.tensor_copy(a3[:, h:h + 1, :],
                                          m3[:, 2 * h:2 * h + 1, :])
                n = h + (n % 2)
            while n > 1:
                if n % 2:
                    nc.vector.tensor_tensor(out=a3[:, 0:1, :],
                                            in0=a3[:, 0:1, :],
                                            in1=a3[:, n - 1:n, :],
                                            op=mybir.AluOpType.add)
                    n -= 1
                h = n // 2
                nc.vector.tensor_tensor(out=a3[:, 0:h, :],
                                        in0=a3[:, 0:h, :],
                                        in1=a3[:, h:2 * h, :],
                                        op=mybir.AluOpType.add)
                n = h

            den = spool.tile([P, HEADS], F32, tag="den")
            nc.vector.tensor_scalar_max(
                den[:], a3[:, 0:1, HD:MW].rearrange("p s c -> p (s c)"), den_eps)
            rec = spool.tile([P, HEADS], F32, tag="rec")
            nc.vector.reciprocal(rec[:], den[:])

            nc.vector.tensor_tensor(
                out=ob.rearrange("p (h d) -> p h d", d=D_OUT),
                in0=a3[:, 0:1, 0:HD].rearrange("p s (h d) -> p (s h) d", d=D_OUT),
                in1=rec[:].unsqueeze(2).broadcast_to([P, HEADS, D_OUT]),
                op=mybir.AluOpType.mult)
            nc.vector.tensor_tensor(out=ob, in0=ob, in1=bias_b[:],
                                    op=mybir.AluOpType.add)
            off += D_b

        # one output write for everything: row b*128+p <- out_all[p, b, :]
        nc.sync.dma_start(
            out_d[:].rearrange("(b p) c -> p b c", p=P),
            out_all[:].rearrange("p (b c) -> p b c", c=HD))

    nc.compile()
    return nc


_PROGRAM_CACHE = {}


def run(feat, W, attn_l, attn_r, bias, src, dst):
    global LAST_RESULTS
    feat = np.asarray(feat, dtype=np.float32)
    src = np.asarray(src, dtype=np.int32)
    dst = np.asarray(dst, dtype=np.int32)

    host = build_host_data(feat, src, dst)

    key = (host["D"], host["S"])
    if key not in _PROGRAM_CACHE:
        _PROGRAM_CACHE[key] = build_program(list(host["D"]), host["S"])
    nc = _PROGRAM_CACHE[key]

    in_maps = []
    for c in range(N_CORES):
        in_maps.append({
            "featT": host["featT"],
            "W": np.asarray(W, dtype=np.float32),
            "attn_l": np.asarray(attn_l, dtype=np.float32),
            "attn_r": np.asarray(attn_r, dtype=np.float32),
            "bias": np.asarray(bias, dtype=np.float32),
            "tab": host["tabs"][c],
        })

    res = run_bass_kernel_spmd(nc, in_maps, core_ids=list(range(N_CORES)))
    LAST_RESULTS = res
    outs = np.stack([res.results[c]["out"] for c in range(N_CORES)], axis=0)
    return outs[host["out_core"], host["out_row"]]


def kernel(feat, W, attn_l, attn_r, bias, src, dst):
    out = run(feat, W, attn_l, attn_r, bias, src, dst)
    return out.reshape(N_NODES, HEADS, D_OUT).astype(np.float32)


# revision 16
# speedup vs baseline: 1.1710x; 1.0165x over previous
"""nn_GATLayer Trainium2 kernel: 8-core SPMD Bass/Tile implementation.

kernel(**inputs) takes the FULL inputs (feat [100000,128] f32, W, attn_l,
attn_r, bias, src/dst [1600000] i32) and returns the FULL output
[100000, 4, 16] f32.

Strategy (dst-sharded, collective-free, node-partitioned edge slots):
  Host: sort nodes by in-degree, deal them round-robin into 98 "levels" x
  8 cores x 128 partition slots so every core sees the same per-level max
  in-degree D_l. Per core, build a slot table [128, sum(D_l)] of h_aug row
  indices: column off_l+s, partition p holds the source row of the s-th
  incoming edge of the node at (level l, partition p); empty slots point
  at a pad row whose el is -30 (exp ~ 0). All float math runs on device.
  Device (per core, same SPMD program, fully unrolled for overlap):
    Phase A: h_aug = feat @ [W | wl | wr] for ALL nodes (replicated),
      where el = feat@wl, er = feat@wr fold the attention dot-products
      into the projection matmul. Rows [h(64) | el(4) | er(4)] bf16,
      stored swizzled (node at featT col c -> row (c%128)*792 + c//128)
      so phase A writes are 2304B-contiguous per partition.
    Phase B (per level, python-unrolled):
      per-slot indirect DMAs gather the block's 128*D source rows
      [128, D, 72] (one offset per partition per instruction); er of the
      block's own nodes comes from one batched upfront DMA; logits =
      leaky_relu(el_src + er_dst) in f32; ex = exp -> bf16 alongside
      msg = h_src * ex; a halving add-tree over the slot axis (f32
      accumulation) produces numerator [128,64] and denominator [128,4]
      in one pass; out = num/max(den,eps)+bias into an SBUF staging
      tile, written to DRAM once at the end.
  Softmax max-subtraction is skipped: alpha = ex/sum(ex) is shift-
  invariant and logits are bounded (~|10|) for this distribution.
"""

import numpy as np
from contextlib import ExitStack

import ml_dtypes

import concourse.bass as bass
import concourse.tile as tile
from concourse import bacc, mybir
from concourse.bass import ds
from concourse.bass_utils import run_bass_kernel_spmd

F32 = mybir.dt.float32
BF16 = mybir.dt.bfloat16
I32 = mybir.dt.int32

P = 128
N_CORES = 8
NB = 98                    # destination levels (blocks per core)
D_OUT = 16
HEADS = 4
HD = HEADS * D_OUT         # 64
HAUG = HD + 2 * HEADS      # 72

NPC = NB * P               # 12544 nodes per core
NPOS = N_CORES * NPC       # 100352 padded node positions
JW = 792                   # h_aug j-columns per partition (784 used + pad)
PAD_ROW = 784              # h_aug row of the pad row (p=0, j=784)
NROWS = P * JW             # 101376 h_aug rows

N_NODES = 100000
IN_DIM = 128

LAST_RESULTS = None        # BassKernelResults of the most recent run


def build_host_data(feat, src, dst):
    """Degree-sorted node layout + per-core slot tables (int math only)."""
    deg = np.zeros(NPOS, dtype=np.int64)
    np.add.at(deg, dst, 1)
    order = np.argsort(-deg, kind="stable")      # rank -> node position
    rank = np.empty(NPOS, dtype=np.int64)
    rank[order] = np.arange(NPOS)                # node -> rank (= featT col)

    # per-level slot width: max degree in level = degree of first rank
    D = deg[order[::P * N_CORES]].astype(np.int64)          # [NB]
    assert D.shape == (NB,)
    off = np.concatenate([[0], np.cumsum(D)])               # [NB+1]
    S = int(off[-1])

    # featT [128, NPOS] bf16, col = rank of node
    featT = np.zeros((IN_DIM, NPOS), dtype=ml_dtypes.bfloat16)
    featT[:, rank[:feat.shape[0]]] = feat.T.astype(ml_dtypes.bfloat16)

    r_src = rank[src]
    r_dst = rank[dst]
    # h_aug row of a node at featT col c: (c % 128)*JW + c//128
    idx_src = ((r_src % P) * JW + r_src // P).astype(np.int32)

    lvl = r_dst // (P * N_CORES)
    core = (r_dst % (P * N_CORES)) // P
    part = r_dst % P

    # slot index within the destination node: stable order among its edges
    e_order = np.argsort(r_dst, kind="stable")
    cnt = np.bincount(r_dst, minlength=NPOS)
    starts = np.concatenate([[0], np.cumsum(cnt)[:-1]])
    slot = np.empty(len(src), dtype=np.int64)
    slot[e_order] = np.arange(len(src)) - starts[r_dst[e_order]]

    col = off[lvl] + slot
    tabs = []
    for c in range(N_CORES):
        m = core == c
        tab = np.full((P, S), PAD_ROW, dtype=np.int32)
        tab[part[m], col[m]] = idx_src[m]
        tabs.append(tab)

    # output mapping: node n -> (core, row l*128+p)
    out_core = ((rank[:N_NODES] % (P * N_CORES)) // P).astype(np.int64)
    out_row = ((rank[:N_NODES] // (P * N_CORES)) * P
               + rank[:N_NODES] % P).astype(np.int64)

    return dict(featT=featT, tabs=tabs, D=tuple(int(x) for x in D),
                out_core=out_core, out_row=out_row, S=S)


def build_program(D, S, den_eps=1e-6):
    """D: per-level slot counts (same for all cores); S = sum(D)."""
    DCAP = max(max(D), 1)
    nc = bacc.Bacc("TRN2", target_bir_lowering=False, debug=False,
                   num_devices=N_CORES)

    featT_d = nc.dram_tensor("featT", [IN_DIM, NPOS], BF16, kind="ExternalInput")
    w_d = nc.dram_tensor("W", [IN_DIM, HD], F32, kind="ExternalInput")
    al_d = nc.dram_tensor("attn_l", [HEADS, D_OUT], F32, kind="ExternalInput")
    ar_d = nc.dram_tensor("attn_r", [HEADS, D_OUT], F32, kind="ExternalInput")
    bias_d = nc.dram_tensor("bias", [HD], F32, kind="ExternalInput")
    tab_d = nc.dram_tensor("tab", [P, S], I32, kind="ExternalInput")

    hA_d = nc.dram_tensor("h_aug", [NROWS, HAUG], BF16, kind="Internal")
    out_d = nc.dram_tensor("out", [NPC, HD], F32, kind="ExternalOutput")

    hA_w = hA_d[:].rearrange("(p j) c -> p (j c)", j=JW)     # [128, JW*72]
    # [p, b, (k c)] view: row p*792 + 8b + k  (b < 99)
    hA_r = hA_d[:].rearrange("(p b k) c -> p b (k c)", b=JW // 8, k=8)

    with tile.TileContext(nc) as tc, ExitStack() as ctx:
        cpool = ctx.enter_context(tc.tile_pool(name="const", bufs=1))
        fpool = ctx.enter_context(tc.tile_pool(name="ft", bufs=3))
        hpool = ctx.enter_context(tc.tile_pool(name="hb", bufs=3))
        pspool = ctx.enter_context(tc.tile_pool(name="psA", bufs=4, space="PSUM"))
        epool = ctx.enter_context(tc.tile_pool(name="edge", bufs=3))
        mpool = ctx.enter_context(tc.tile_pool(name="msg", bufs=2))
        apool = ctx.enter_context(tc.tile_pool(name="acc", bufs=2))
        spool = ctx.enter_context(tc.tile_pool(name="small", bufs=4))

        pid = nc.sync.partition_id()

        # ---- constants / parameter prep (one-time) ----
        ones_row = cpool.tile([1, P], F32)
        nc.vector.memset(ones_row[:], 1.0)

        def pe_broadcast(row_ap, width):
            ps = pspool.tile([P, width], F32)
            nc.tensor.matmul(out=ps[:], lhsT=ones_row[:], rhs=row_ap,
                             start=True, stop=True)
            t = cpool.tile([P, width], F32)
            nc.vector.tensor_copy(t[:], ps[:])
            return t

        al_row = cpool.tile([1, HD], F32)
        nc.sync.dma_start(al_row[:], al_d[:].rearrange("h d -> (h d)").unsqueeze(0))
        ar_row = cpool.tile([1, HD], F32)
        nc.sync.dma_start(ar_row[:], ar_d[:].rearrange("h d -> (h d)").unsqueeze(0))
        b_row = cpool.tile([1, HD], F32)
        nc.sync.dma_start(b_row[:], bias_d[:].unsqueeze(0))

        al_b = pe_broadcast(al_row[:], HD)
        ar_b = pe_broadcast(ar_row[:], HD)
        bias_b = pe_broadcast(b_row[:], HD)

        w_aug = cpool.tile([P, HAUG], F32)
        nc.sync.dma_start(w_aug[:, 0:HD], w_d[:, :])
        tmp = cpool.tile([P, HD], F32)
        nc.vector.tensor_tensor(out=tmp[:], in0=w_aug[:, 0:HD], in1=al_b[:],
                                op=mybir.AluOpType.mult)
        nc.vector.tensor_reduce(
            out=w_aug[:, HD:HD + HEADS],
            in_=tmp[:].rearrange("p (h d) -> p h d", d=D_OUT),
            axis=mybir.AxisListType.X, op=mybir.AluOpType.add)
        nc.vector.tensor_tensor(out=tmp[:], in0=w_aug[:, 0:HD], in1=ar_b[:],
                                op=mybir.AluOpType.mult)
        nc.vector.tensor_reduce(
            out=w_aug[:, HD + HEADS:HAUG],
            in_=tmp[:].rearrange("p (h d) -> p h d", d=D_OUT),
            axis=mybir.AxisListType.X, op=mybir.AluOpType.add)
        w_augb = cpool.tile([P, HAUG], BF16)
        nc.vector.tensor_copy(w_augb[:], w_aug[:])

        # pad row: h = 0, el = -30 (exp -> ~0), er = 0
        pad_t = cpool.tile([1, HAUG], BF16)
        nc.vector.memset(pad_t[:], 0.0)
        nc.vector.memset(pad_t[:, HD:HD + HEADS], -30.0)
        nc.sync.dma_start(hA_d[ds(PAD_ROW, 1), :], pad_t[:])

        # whole slot table staged once (offsets into it are compile-time)
        stab = cpool.tile([P, S], I32)
        nc.sync.dma_start(stab[:], tab_d[:, :])

        # ---- Phase A: projection (replicated over all nodes) ----
        PROJ_G = 16
        n_groups = NPOS // (PROJ_G * P)          # 49
        for gi in range(n_groups):
            ft = fpool.tile([P, PROJ_G * P], BF16, tag="ft")
            nc.sync.dma_start(ft[:], featT_d[:, ds(gi * (PROJ_G * P), PROJ_G * P)])
            hb = hpool.tile([P, PROJ_G * HAUG], BF16, tag="hb")
            for j in range(PROJ_G):
                ps = pspool.tile([P, HAUG], F32)
                nc.tensor.matmul(out=ps[:], lhsT=ft[:, j * P:(j + 1) * P],
                                 rhs=w_augb[:], start=True, stop=True)
                # split psum->sbuf cast copies across Act and DVE so the
                # copy chain doesn't serialize phase A on one engine
                if j % 2 == 0:
                    nc.scalar.copy(hb[:, j * HAUG:(j + 1) * HAUG], ps[:])
                else:
                    nc.vector.tensor_copy(hb[:, j * HAUG:(j + 1) * HAUG], ps[:])
            nc.sync.dma_start(hA_w[:, ds(gi * (PROJ_G * HAUG), PROJ_G * HAUG)],
                              hb[:])

        tc.strict_bb_all_engine_barrier()

        # er of this core's nodes for ALL levels in one DMA: [128, 98*4]
        er_all = cpool.tile([P, NB * HEADS], BF16)
        nc.sync.dma_start(
            er_all[:].rearrange("p (b c) -> p b c", c=HEADS),
            hA_r[:, 0:NB, ds(pid * HAUG + HD + HEADS, HEADS)])

        # output staging: all levels, one DRAM write at the end
        out_all = cpool.tile([P, NB * HD], F32)

        # ---- Phase B: per-level edge aggregation (fully unrolled) ----
        off = 0
        for b in range(NB):
            D_b = D[b]
            ob = out_all[:, b * HD:(b + 1) * HD]
            if D_b == 0:
                nc.vector.tensor_copy(ob, bias_b[:])
                continue

            hsrc = epool.tile([P, DCAP * HAUG], BF16, tag="hsrc")
            for dd in range(D_b):
                nc.gpsimd.indirect_dma_start(
                    out=hsrc[:, dd * HAUG:(dd + 1) * HAUG],
                    out_offset=None,
                    in_=hA_d[:],
                    in_offset=bass.IndirectOffsetOnAxis(
                        ap=stab[:, off + dd:off + dd + 1], axis=0),
                )
            h3 = hsrc[:].rearrange("p (s c) -> p s c", c=HAUG)

            lg = spool.tile([P, DCAP * HEADS], F32, tag="lg")
            lg3 = lg[:].rearrange("p (s h) -> p s h", h=HEADS)
            nc.vector.tensor_tensor(
                out=lg3[:, 0:D_b, :],
                in0=h3[:, 0:D_b, HD:HD + HEADS],
                in1=er_all[:, b * HEADS:(b + 1) * HEADS]
                    .unsqueeze(1).broadcast_to([P, D_b, HEADS]),
                op=mybir.AluOpType.add)
            lk = spool.tile([P, DCAP * HEADS], F32, tag="lk")
            nc.vector.tensor_scalar_mul(lk[:, 0:D_b * HEADS],
                                        lg[:, 0:D_b * HEADS], 0.2)
            nc.vector.tensor_tensor(out=lk[:, 0:D_b * HEADS],
                                    in0=lk[:, 0:D_b * HEADS],
                                    in1=lg[:, 0:D_b * HEADS],
                                    op=mybir.AluOpType.max)

            MW = HD + HEADS   # 68
            msg = mpool.tile([P, DCAP * MW], BF16, tag="msg")
            m3 = msg[:].rearrange("p (s c) -> p s c", c=MW)
            nc.scalar.activation(
                out=m3[:, 0:D_b, HD:MW],
                in_=lk[:, 0:D_b * HEADS].rearrange("p (s h) -> p s h", h=HEADS),
                func=mybir.ActivationFunctionType.Exp)
            nc.vector.tensor_tensor(
                out=m3[:, 0:D_b, 0:HD].rearrange("p s (h d) -> p s h d", d=D_OUT),
                in0=h3[:, 0:D_b, 0:HD].rearrange("p s (h d) -> p s h d", d=D_OUT),
                in1=m3[:, 0:D_b, HD:MW].unsqueeze(3).broadcast_to(
                    [P, D_b, HEADS, D_OUT]),
                op=mybir.AluOpType.mult)

            # halving add-tree over slots: bf16 msg -> f32 acc
            accf = apool.tile([P, ((DCAP + 1) // 2) * MW], F32, tag="accf")
            a3 = accf[:].rearrange("p (s c) -> p s c", c=MW)
            n = D_b
            if n == 1:
                nc.vector.tensor_copy(a3[:, 0:1, :], m3[:, 0:1, :])
            else:
                h = n // 2
                nc.vector.tensor_tensor(out=a3[:, 0:h, :],
                                        in0=m3[:, 0:h, :],
                                        in1=m3[:, h:2 * h, :],
                                        op=mybir.AluOpType.add)
                if n % 2:
                    nc.vector.tensor_copy(a3[:, h:h + 1, :],
                                          m3[:, 2 * h:2 * h + 1, :])
                n = h + (n % 2)
            while n > 1:
                if n % 2:
                    nc.vector.tensor_tensor(out=a3[:, 0:1, :],
                                            in0=a3[:, 0:1, :],
                                            in1=a3[:, n - 1:n, :],
                                            op=mybir.AluOpType.add)
                    n -= 1
                h = n // 2
                nc.vector.tensor_tensor(out=a3[:, 0:h, :],
                                        in0=a3[:, 0:h, :],
                                        in1=a3[:, h:2 * h, :],
                                        op=mybir.AluOpType.add)
                n = h

            den = spool.tile([P, HEADS], F32, tag="den")
            nc.vector.tensor_scalar_max(
                den[:], a3[:, 0:1, HD:MW].rearrange("p s c -> p (s c)"), den_eps)
            rec = spool.tile([P, HEADS], F32, tag="rec")
            nc.vector.reciprocal(rec[:], den[:])

            nc.vector.tensor_tensor(
                out=ob.rearrange("p (h d) -> p h d", d=D_OUT),
                in0=a3[:, 0:1, 0:HD].rearrange("p s (h d) -> p (s h) d", d=D_OUT),
                in1=rec[:].unsqueeze(2).broadcast_to([P, HEADS, D_OUT]),
                op=mybir.AluOpType.mult)
            nc.vector.tensor_tensor(out=ob, in0=ob, in1=bias_b[:],
                                    op=mybir.AluOpType.add)
            off += D_b

        # one output write for everything: row b*128+p <- out_all[p, b, :]
        nc.sync.dma_start(
            out_d[:].rearrange("(b p) c -> p b c", p=P),
            out_all[:].rearrange("p (b c) -> p b c", c=HD))

    nc.compile()
    return nc


_PROGRAM_CACHE = {}


def run(feat, W, attn_l, attn_r, bias, src, dst):
    global LAST_RESULTS
    feat = np.asarray(feat, dtype=np.float32)
    src = np.asarray(src, dtype=np.int32)
    dst = np.asarray(dst, dtype=np.int32)

    host = build_host_data(feat, src, dst)

    key = (host["D"], host["S"])
    if key not in _PROGRAM_CACHE:
        _PROGRAM_CACHE[key] = build_program(list(host["D"]), host["S"])
    nc = _PROGRAM_CACHE[key]

    in_maps = []
    for c in range(N_CORES):
        in_maps.append({
            "featT": host["featT"],
            "W": np.asarray(W, dtype=np.float32),
            "attn_l": np.asarray(attn_l, dtype=np.float32),
            "attn_r": np.asarray(attn_r, dtype=np.float32),
            "bias": np.asarray(bias, dtype=np.float32),
            "tab": host["tabs"][c],
        })

    res = run_bass_kernel_spmd(nc, in_maps, core_ids=list(range(N_CORES)))
    LAST_RESULTS = res
    outs = np.stack([res.results[c]["out"] for c in range(N_CORES)], axis=0)
    return outs[host["out_core"], host["out_row"]]


def kernel(feat, W, attn_l, attn_r, bias, src, dst):
    out = run(feat, W, attn_l, attn_r, bias, src, dst)
    return out.reshape(N_NODES, HEADS, D_OUT).astype(np.float32)
